# revision 6
# baseline (speedup 1.0000x reference)
"""Cosine-similarity retrieval kernel for 8 Trainium2 NeuronCores.

Computes out[n, m] = <x1[n]/||x1[n]||, x2[m]/||x2[m]||> / TEMP for
x1, x2 of shape (8192, 1024) fp32 (output (8192, 8192) fp32).

Sharding: x1 rows data-parallel across the 8 cores (1024-row slabs),
x2 replicated. Each core computes its (1024, 8192) slab of the score
matrix.

Device pipeline (per core), all arithmetic on-device:
  - GEMM operands are uploaded d-major (host transpose only, no host
    math): x1t [d, n_slab], x2t [d, m] fp32; SWDGE DMA casts f32->bf16
    on the way into SBUF
  - x1 row norms: a natural-layout copy of the x1 slab is squared on
    ACT with accum_out, giving per-partition sum(x1^2) directly; then
    Sqrt(. * TEMP^2) + reciprocal_approx_fast -> n1i = (1/TEMP)/||x1||
    laid out per-partition, matching the output tiles' row axis
  - x2 row norms: ones-matmul column sums of ACT-squared bf16 tiles
    (the [128,128] ones stationary operand replicates the column sums
    across all partitions for free), then Sqrt + reciprocal_approx_fast
    -> srep2 = 1/||x2|| replicated across partitions
  - main GEMM: bf16 matmuls, k-accumulated in PSUM, N=512 chunks; the
    PSUM->SBUF drain applies both scales in one fused DVE op:
    out = (psum * n1i) * srep2  (scalar_tensor_tensor)
"""

import sys

if "/opt/trn_rl_repo" not in sys.path:
    sys.path.insert(0, "/opt/trn_rl_repo")

import numpy as np

TEMP = 0.05
N_CORES = 8

_CACHE = {}


def _build(n_slab, m, d):
    """Build + compile the per-core Bass kernel. Shapes are per-core."""
    from contextlib import ExitStack

    import concourse.mybir as mybir
    import concourse.tile as tile
    from concourse import bacc

    f32 = mybir.dt.float32
    bf16 = mybir.dt.bfloat16
    AF = mybir.ActivationFunctionType
    MUL = mybir.AluOpType.mult

    assert d % 128 == 0 and n_slab % 128 == 0 and m % 1024 == 0
    KT = d // 128          # contraction k-tiles
    NMT = n_slab // 128    # output row tiles
    CB = 1024              # x2 column block processed per stage-B step
    NCB = m // CB
    CHW = 512              # psum chunk width (one PSUM bank)

    nc = bacc.Bacc("TRN2", target_bir_lowering=False, debug=False,
                   num_devices=N_CORES)
    x1nat = nc.declare_dram_parameter("x1nat", [n_slab, d], f32, isOutput=False)
    x1t = nc.declare_dram_parameter("x1t", [d, n_slab], f32, isOutput=False)
    x2t = nc.declare_dram_parameter("x2t", [d, m], f32, isOutput=False)
    out = nc.declare_dram_parameter("out", [n_slab, m], f32, isOutput=True)

    x1nat_t = x1nat.ap().rearrange("(t p) dd -> t p dd", p=128)
    x1t_k = x1t.ap().rearrange("(kk p) n -> kk p n", p=128)
    x2t_k = x2t.ap().rearrange("(kk p) mm -> kk p mm", p=128)
    out_ap = out.ap()

    with tile.TileContext(nc) as tc, ExitStack() as ctx:
        resid = ctx.enter_context(tc.tile_pool(name="resid", bufs=1))
        x2n = resid.tile([128, KT, m], bf16)        # bf16 cast of x2t
        x1n = resid.tile([128, KT, n_slab], bf16)   # bf16 cast of x1t
        srep2 = resid.tile([128, m], f32)           # 1/||x2|| replicated
        n1i = resid.tile([128, NMT], f32)           # (1/TEMP)/||x1|| per-partition
        ones = resid.tile([128, 128], bf16)
        nc.vector.memset(ones, 1.0)

        # 2 banks for norm accumulation, rest for the main GEMM
        normp = ctx.enter_context(tc.tile_pool(name="normp", bufs=1, space="PSUM"))
        vec = ctx.enter_context(tc.tile_pool(name="vec", bufs=2))

        # ---------------- stage A: x1 prep ----------------
        # bf16 GEMM operand (cast in DMA)
        for k in range(KT):
            nc.gpsimd.dma_start(out=x1n[:, k, :], in_=x1t_k[k])
        # per-partition row norms from the natural-layout slab
        with tc.tile_pool(name="a_in", bufs=3) as a_in, \
             tc.tile_pool(name="a_sq", bufs=2) as a_sq:
            n1sq = vec.tile([128, NMT], f32, tag="n1sq", name="n1sq", bufs=1)
            for mt in range(NMT):
                xf = a_in.tile([128, d], f32, tag="a_xf", name="a_xf")
                nc.sync.dma_start(out=xf[:], in_=x1nat_t[mt])
                sq = a_sq.tile([128, d], bf16, tag="a_sq", name="a_sqt")
                nc.scalar.activation(sq[:], xf[:], AF.Square,
                                     accum_out=n1sq[:, mt:mt + 1])
            tmp = vec.tile([128, NMT], f32, tag="n1tmp", name="n1tmp", bufs=1)
            # sqrt(nsq * TEMP^2) = ||x1|| * TEMP ; reciprocal -> (1/TEMP)/||x1||
            nc.scalar.activation(tmp[:], n1sq[:], AF.Sqrt,
                                 scale=float(TEMP * TEMP))
            nc.vector.reciprocal_approx_fast(out=n1i[:], in_=tmp[:])

        # ------------- stages B+C interleaved over column blocks -------------
        with tc.tile_pool(name="b_sq", bufs=2) as b_sq, \
             tc.tile_pool(name="cps", bufs=6, space="PSUM") as cps, \
             tc.tile_pool(name="ost", bufs=2) as ost:
            for cb in range(NCB):
                csl = slice(cb * CB, (cb + 1) * CB)
                # -- stage B: cast-DMA + norms for this column block
                npsb = [normp.tile([128, CHW], f32, tag=f"np{i}", name=f"npsB{i}")
                        for i in range(CB // CHW)]
                for k in range(KT):
                    nc.gpsimd.dma_start(out=x2n[:, k, csl], in_=x2t_k[k][:, csl])
                    sq = b_sq.tile([128, CB], bf16, tag="b_sq", name="b_sqt")
                    nc.scalar.activation(sq[:], x2n[:, k, csl], AF.Square)
                    for i in range(CB // CHW):
                        nc.tensor.matmul(npsb[i][:], ones[:, :128],
                                         sq[:, i * CHW:(i + 1) * CHW],
                                         start=(k == 0), stop=(k == KT - 1))
                for i in range(CB // CHW):
                    off = cb * CB + i * CHW
                    tmp = vec.tile([128, CHW], f32, tag="vtmp", name="b_tmp")
                    nc.scalar.activation(tmp[:], npsb[i][:], AF.Sqrt)
                    nc.vector.reciprocal_approx_fast(out=srep2[:, off:off + CHW],
                                                     in_=tmp[:])
                # -- stage C: output tiles of this column block
                for mt in range(NMT):
                    pss = [cps.tile([128, CHW], f32, tag="c_ps", name="c_ps")
                           for _ in range(CB // CHW)]
                    for i, ps in enumerate(pss):
                        c0 = cb * CB + i * CHW
                        for k in range(KT):
                            nc.tensor.matmul(
                                ps[:],
                                x1n[:, k, mt * 128:(mt + 1) * 128],
                                x2n[:, k, c0:c0 + CHW],
                                start=(k == 0), stop=(k == KT - 1))
                    ot = ost.tile([128, CB], f32, tag="c_ot", name="c_ot")
                    for i, ps in enumerate(pss):
                        c0 = cb * CB + i * CHW
                        # out = (psum * n1i_row) * srep2_col  — both scales fused
                        nc.vector.scalar_tensor_tensor(
                            out=ot[:, i * CHW:(i + 1) * CHW],
                            in0=ps[:],
                            scalar=n1i[:, mt:mt + 1],
                            in1=srep2[:, c0:c0 + CHW],
                            op0=MUL, op1=MUL)
                    nc.sync.dma_start(
                        out=out_ap[mt * 128:(mt + 1) * 128, csl], in_=ot[:])

    nc.compile()
    return nc


def _get_nc(n_slab, m, d):
    key = (n_slab, m, d)
    if key not in _CACHE:
        _CACHE[key] = _build(n_slab, m, d)
    return _CACHE[key]


def kernel(x1, x2):
    from concourse.bass_utils import run_bass_kernel_spmd

    x1 = np.asarray(x1, dtype=np.float32)
    x2 = np.asarray(x2, dtype=np.float32)
    n, d = x1.shape
    m, d2 = x2.shape
    assert d == d2 and n % N_CORES == 0
    n_slab = n // N_CORES

    nc = _get_nc(n_slab, m, d)

    x1t = np.ascontiguousarray(x1.T)  # [d, n]
    x2t = np.ascontiguousarray(x2.T)  # [d, m]
    in_maps = [
        {"x1nat": np.ascontiguousarray(x1[i * n_slab:(i + 1) * n_slab]),
         "x1t": np.ascontiguousarray(x1t[:, i * n_slab:(i + 1) * n_slab]),
         "x2t": x2t}
        for i in range(N_CORES)
    ]
    res = run_bass_kernel_spmd(nc, in_maps, core_ids=list(range(N_CORES)))
    return np.concatenate([res.results[i]["out"] for i in range(N_CORES)], axis=0)


if __name__ == "__main__":
    # small-shape self test
    rng = np.random.default_rng(0)
    n, m, d = 1024, 2048, 256
    x1 = rng.standard_normal((n, d), dtype=np.float32)
    x2 = rng.standard_normal((m, d), dtype=np.float32)
    got = kernel(x1, x2)
    x1n = x1 / np.linalg.norm(x1, axis=1, keepdims=True)
    x2n = x2 / np.linalg.norm(x2, axis=1, keepdims=True)
    want = (x1n @ x2n.T) / TEMP
    rel = np.linalg.norm(got - want) / np.linalg.norm(want)
    print("rel l2 err:", rel)
    print("max abs err:", np.abs(got - want).max(), "scale:", np.abs(want).max())


# revision 7
# speedup vs baseline: 1.0216x; 1.0216x over previous
"""Cosine-similarity retrieval kernel for 8 Trainium2 NeuronCores.

Computes out[n, m] = <x1[n]/||x1[n]||, x2[m]/||x2[m]||> / TEMP for
x1, x2 of shape (8192, 1024) fp32 (output (8192, 8192) fp32).

Sharding: x1 rows data-parallel across the 8 cores (1024-row slabs),
x2 replicated. Each core computes its (1024, 8192) slab of the score
matrix.

Device pipeline (per core), all arithmetic on-device:
  - inputs are uploaded d-major (host transpose only, no host math):
    x1t [d, n_slab], x2t [d, m] fp32; SWDGE DMA casts f32->bf16 on the
    way into SBUF
  - row norms of the bf16 data via ones-matmul column sums of
    ACT-squared tiles (the [128,128] ones stationary operand replicates
    the column sums across all partitions for free), then Sqrt (ACT) +
    reciprocal_approx_fast (DVE)
  - x1 is pre-scaled by (1/TEMP)/||x1|| into bf16 x1n (keeps the PE
    busy from the very start of the kernel via its norm matmuls)
  - x2's 1/||x2|| column scale is applied to the PSUM result during the
    PSUM->SBUF drain (DVE tensor_mul)
  - main GEMM: bf16 matmuls, k-accumulated in PSUM, N=512 chunks
"""

import sys

if "/opt/trn_rl_repo" not in sys.path:
    sys.path.insert(0, "/opt/trn_rl_repo")

import numpy as np

TEMP = 0.05
N_CORES = 8

_CACHE = {}


def _ceil_div(a, b):
    return (a + b - 1) // b


def _build(n_slab, m, d):
    """Build + compile the per-core Bass kernel. Shapes are per-core."""
    from contextlib import ExitStack

    import concourse.mybir as mybir
    import concourse.tile as tile
    from concourse import bacc

    f32 = mybir.dt.float32
    bf16 = mybir.dt.bfloat16
    AF = mybir.ActivationFunctionType

    assert d % 128 == 0 and n_slab % 128 == 0 and m % 1024 == 0
    KT = d // 128          # contraction k-tiles
    NMT = n_slab // 128    # output row tiles
    CB = 1024              # x2 column block processed per stage-B step
    NCB = m // CB
    CHW = 512              # psum chunk width (one PSUM bank)
    a_chunks = [(i * CHW, min(CHW, n_slab - i * CHW)) for i in range(_ceil_div(n_slab, CHW))]

    nc = bacc.Bacc("TRN2", target_bir_lowering=False, debug=False,
                   num_devices=N_CORES)
    x1t = nc.declare_dram_parameter("x1t", [d, n_slab], f32, isOutput=False)
    x2t = nc.declare_dram_parameter("x2t", [d, m], f32, isOutput=False)
    out = nc.declare_dram_parameter("out", [n_slab, m], f32, isOutput=True)

    x1t_k = x1t.ap().rearrange("(kk p) n -> kk p n", p=128)
    x2t_k = x2t.ap().rearrange("(kk p) mm -> kk p mm", p=128)
    out_ap = out.ap()

    with tile.TileContext(nc) as tc, ExitStack() as ctx:
        resid = ctx.enter_context(tc.tile_pool(name="resid", bufs=1))
        x2n = resid.tile([128, KT, m], bf16)        # bf16 cast of x2t
        x1n = resid.tile([128, KT, n_slab], bf16)   # pre-scaled x1
        srep2 = resid.tile([128, m], f32)           # 1/||x2|| replicated
        srep1 = resid.tile([128, n_slab], f32)      # (1/TEMP)/||x1|| replicated
        ones = resid.tile([128, 128], bf16)
        nc.vector.memset(ones, 1.0)

        # 2 banks for norm accumulation, rest for the main GEMM
        normp = ctx.enter_context(tc.tile_pool(name="normp", bufs=1, space="PSUM"))
        vec = ctx.enter_context(tc.tile_pool(name="vec", bufs=2))

        # ---------------- stage A: x1 prep ----------------
        with tc.tile_pool(name="a_c", bufs=1) as a_c, \
             tc.tile_pool(name="a_sq", bufs=2) as a_sq:
            x1c = a_c.tile([128, KT, n_slab], bf16)
            nps = [normp.tile([128, w], f32, tag=f"np{i}", name=f"npsA{i}")
                   for i, (_, w) in enumerate(a_chunks)]
            for k in range(KT):
                # SWDGE DMA with inline f32->bf16 cast
                nc.gpsimd.dma_start(out=x1c[:, k, :], in_=x1t_k[k])
                sq = a_sq.tile([128, n_slab], bf16, tag="a_sq", name="a_sqt")
                nc.scalar.activation(sq[:], x1c[:, k, :], AF.Square)
                for i, (off, w) in enumerate(a_chunks):
                    nc.tensor.matmul(nps[i][:], ones[:, :128], sq[:, off:off + w],
                                     start=(k == 0), stop=(k == KT - 1))
            for i, (off, w) in enumerate(a_chunks):
                tmp = vec.tile([128, CHW], f32, tag="vtmp", name="a_tmp")
                # sqrt(nsq * TEMP^2) = ||x1|| * TEMP ; reciprocal -> (1/TEMP)/||x1||
                nc.scalar.activation(tmp[:, :w], nps[i][:], AF.Sqrt,
                                     scale=float(TEMP * TEMP))
                nc.vector.reciprocal_approx_fast(out=srep1[:, off:off + w],
                                                 in_=tmp[:, :w])
                # chunk-major so early output row-tiles unblock asap
                for k in range(KT):
                    nc.vector.tensor_mul(x1n[:, k, off:off + w],
                                         x1c[:, k, off:off + w],
                                         srep1[:, off:off + w])

        # ------------- stages B+C interleaved over column blocks -------------
        with tc.tile_pool(name="b_sq", bufs=2) as b_sq, \
             tc.tile_pool(name="cps", bufs=6, space="PSUM") as cps, \
             tc.tile_pool(name="ost", bufs=2) as ost:
            for cb in range(NCB):
                csl = slice(cb * CB, (cb + 1) * CB)
                # -- stage B: cast-DMA + norms for this column block
                npsb = [normp.tile([128, CHW], f32, tag=f"np{i}", name=f"npsB{i}")
                        for i in range(CB // CHW)]
                for k in range(KT):
                    nc.gpsimd.dma_start(out=x2n[:, k, csl], in_=x2t_k[k][:, csl])
                    sq = b_sq.tile([128, CB], bf16, tag="b_sq", name="b_sqt")
                    nc.scalar.activation(sq[:], x2n[:, k, csl], AF.Square)
                    for i in range(CB // CHW):
                        nc.tensor.matmul(npsb[i][:], ones[:, :128],
                                         sq[:, i * CHW:(i + 1) * CHW],
                                         start=(k == 0), stop=(k == KT - 1))
                for i in range(CB // CHW):
                    off = cb * CB + i * CHW
                    tmp = vec.tile([128, CHW], f32, tag="vtmp", name="b_tmp")
                    nc.scalar.activation(tmp[:], npsb[i][:], AF.Sqrt)
                    nc.vector.reciprocal_approx_fast(out=srep2[:, off:off + CHW],
                                                     in_=tmp[:])
                # -- stage C: output tiles of this column block
                for mt in range(NMT):
                    pss = [cps.tile([128, CHW], f32, tag="c_ps", name="c_ps")
                           for _ in range(CB // CHW)]
                    for i, ps in enumerate(pss):
                        c0 = cb * CB + i * CHW
                        for k in range(KT):
                            nc.tensor.matmul(
                                ps[:],
                                x1n[:, k, mt * 128:(mt + 1) * 128],
                                x2n[:, k, c0:c0 + CHW],
                                start=(k == 0), stop=(k == KT - 1))
                    ot = ost.tile([128, CB], f32, tag="c_ot", name="c_ot")
                    for i, ps in enumerate(pss):
                        c0 = cb * CB + i * CHW
                        nc.vector.tensor_mul(ot[:, i * CHW:(i + 1) * CHW], ps[:],
                                             srep2[:, c0:c0 + CHW])
                    nc.sync.dma_start(
                        out=out_ap[mt * 128:(mt + 1) * 128, csl], in_=ot[:])

    nc.compile()
    return nc


def _get_nc(n_slab, m, d):
    key = (n_slab, m, d)
    if key not in _CACHE:
        _CACHE[key] = _build(n_slab, m, d)
    return _CACHE[key]


def _in_maps(x1, x2, n_slab):
    x1t = np.ascontiguousarray(x1.T)  # [d, n]
    x2t = np.ascontiguousarray(x2.T)  # [d, m]
    return [
        {"x1t": np.ascontiguousarray(x1t[:, i * n_slab:(i + 1) * n_slab]),
         "x2t": x2t}
        for i in range(N_CORES)
    ]


def kernel(x1, x2):
    from concourse.bass_utils import run_bass_kernel_spmd

    x1 = np.asarray(x1, dtype=np.float32)
    x2 = np.asarray(x2, dtype=np.float32)
    n, d = x1.shape
    m, d2 = x2.shape
    assert d == d2 and n % N_CORES == 0
    n_slab = n // N_CORES

    nc = _get_nc(n_slab, m, d)
    res = run_bass_kernel_spmd(nc, _in_maps(x1, x2, n_slab),
                               core_ids=list(range(N_CORES)))
    return np.concatenate([res.results[i]["out"] for i in range(N_CORES)], axis=0)


if __name__ == "__main__":
    # small-shape self test
    rng = np.random.default_rng(0)
    n, m, d = 1024, 2048, 256
    x1 = rng.standard_normal((n, d), dtype=np.float32)
    x2 = rng.standard_normal((m, d), dtype=np.float32)
    got = kernel(x1, x2)
    x1n = x1 / np.linalg.norm(x1, axis=1, keepdims=True)
    x2n = x2 / np.linalg.norm(x2, axis=1, keepdims=True)
    want = (x1n @ x2n.T) / TEMP
    rel = np.linalg.norm(got - want) / np.linalg.norm(want)
    print("rel l2 err:", rel)
    print("max abs err:", np.abs(got - want).max(), "scale:", np.abs(want).max())
